# revision 15
# baseline (speedup 1.0000x reference)
"""TRN2 Bass kernel for nn_MultiHeadSelfAttention (B=2, S=2048, D=1024, H=16).

Sharding: 8 NeuronCores = 2 batches x 4 head-groups (4 heads each).
Each core computes its batch's Q/K/V projections for its 4 heads, the
attention, and a partial output projection; the host sums the 4 partials
per batch and adds b_o.

Per-core pipeline (all matmuls in float32r = TF32-like, full PE rate):
  - host feeds x[b].T and pre-transposed weight slices (no on-chip transposes)
  - QT/KT projections land with head-dim on partitions; V lands natural
    [s, head, 64] with a fused ones column (gives softmax denominators free)
  - scores computed transposed [keys, q] per 128-key block; exp on ScalarE
    with the 1/sqrt(64) scale fused; no max subtraction (scores bounded ~9)
  - attnU^T [65, S] accumulates over key blocks in PSUM; row 64 = denominator
  - normalize via reciprocal + DRAM-bounce partition broadcast
  - output projection accumulates all 4 heads per PSUM bank
"""

import hashlib
import os
import shutil
from contextlib import ExitStack

import numpy as np

import concourse.bass as bass
import concourse.bacc as bacc
import concourse.mybir as mybir
import concourse.tile as tile
from concourse import bass_utils, bass2jax
from concourse.bass_utils import run_bass_kernel_spmd

F32 = mybir.dt.float32
F32R = mybir.dt.float32r
AF = mybir.ActivationFunctionType

B, S, D, NH, HD = 2, 2048, 1024, 16, 64
N_CORES = 8
HL = NH // (N_CORES // B)  # 4 local heads per core
HP = HL * HD               # 256 local head dims

_NEFF_CACHE = os.path.expanduser("~/.cache/bass-neff-cache")


def _install_neff_cache():
    """Content-addressed disk cache around compile_bir_kernel (walrus is slow)."""
    if getattr(bass2jax, "_neff_cache_installed", False):
        return
    os.makedirs(_NEFF_CACHE, exist_ok=True)
    orig = bass2jax.compile_bir_kernel

    def cached(bir_json, tmpdir, neff_name="file.neff"):
        key = hashlib.sha256(bir_json).hexdigest()[:32]
        cpath = os.path.join(_NEFF_CACHE, key + ".neff")
        dst = os.path.join(tmpdir, neff_name)
        if os.path.exists(cpath):
            shutil.copy(cpath, dst)
            return dst
        out = orig(bir_json, tmpdir, neff_name=neff_name)
        try:
            shutil.copy(out, cpath)
        except OSError:
            pass
        return out

    bass2jax.compile_bir_kernel = cached
    bass2jax._neff_cache_installed = True


def build_mha_core(S=S, D=D, HL=HL, HD=HD, exp_dtype=F32R):
    """Build + compile the per-core MHA program."""
    HP = HL * HD
    KC = D // 128             # contraction chunks for projections
    KB = S // 128             # key blocks
    ST = S // 128             # s tiles for V projection / output
    QCH = min(1024, S)        # q chunk (scores psum free dim)
    NQH = S // QCH
    QW = min(512, QCH)
    NQQ = QCH // QW
    NPAIR = HP // 128
    SC = max(1, S // 512)
    SCW = min(512, S)

    nc = bacc.Bacc("TRN2", target_bir_lowering=False, debug=False)

    xT = nc.dram_tensor("xT", [D, S], F32R, kind="ExternalInput")
    wqT = nc.dram_tensor("wqT", [D, HP], F32R, kind="ExternalInput")
    wkT = nc.dram_tensor("wkT", [D, HP], F32R, kind="ExternalInput")
    wvT = nc.dram_tensor("wvT", [D, HL * (HD + 1)], F32R, kind="ExternalInput")
    woT = nc.dram_tensor("woT", [HP, D], F32R, kind="ExternalInput")
    bq = nc.dram_tensor("bq", [HP], F32, kind="ExternalInput")
    bk = nc.dram_tensor("bk", [HP], F32, kind="ExternalInput")
    bv = nc.dram_tensor("bv", [HL * (HD + 1)], F32, kind="ExternalInput")
    outP = nc.dram_tensor("outP", [S, D], F32, kind="ExternalOutput")
    den_dram = nc.dram_tensor("den_dram", [HL, S], F32, kind="Internal")
    rcp_dram = nc.dram_tensor("rcp_dram", [HL, S], F32, kind="Internal")

    with tile.TileContext(nc) as tc, ExitStack() as top:
        persist = top.enter_context(tc.tile_pool(name="persist", bufs=1))

        QT = [persist.tile([128, S], F32R, name=f"QT{i}") for i in range(NPAIR)]
        KT = [persist.tile([128, S], F32R, name=f"KT{i}") for i in range(NPAIR)]
        Vt = [persist.tile([128, HL, HD + 1], F32R, name=f"V{i}") for i in range(KB)]
        attnP = [persist.tile([128, S], F32R, name=f"attnP{i}") for i in range(NPAIR)]
        wo_s = [persist.tile([128, D], F32R, name=f"wo{i}") for i in range(NPAIR)]
        bq_sb = persist.tile([128, NPAIR], F32, name="bq_sb")
        bk_sb = persist.tile([128, NPAIR], F32, name="bk_sb")
        bv_sb = persist.tile([128, HL * (HD + 1)], F32, name="bv_sb")

        nc.gpsimd.dma_start(out=bq_sb[:, :], in_=bq.ap().rearrange("(hp p) -> p hp", p=128))
        nc.gpsimd.dma_start(out=bk_sb[:, :], in_=bk.ap().rearrange("(hp p) -> p hp", p=128))
        bv_bc = bass.AP(tensor=bv.ap().tensor, offset=0, ap=[[0, 128], [1, HL * (HD + 1)]])
        nc.gpsimd.dma_start(out=bv_sb[:, :], in_=bv_bc)
        for i in range(NPAIR):
            nc.gpsimd.dma_start(out=wo_s[i][:], in_=woT.ap()[i * 128 : (i + 1) * 128, :])

        # ---------------- phase A: projections ----------------
        # Loads are emitted ic-interleaved and split across both HWDGE engines
        # (SP + ACT) so projection matmuls start as soon as chunk 0 lands.
        # V is projected first (attention's kb loop needs every V tile), then
        # the Q/K pair-0 tiles so the first attention block starts early.
        with ExitStack() as phA:
            xpool = phA.enter_context(tc.tile_pool(name="xTp", bufs=1))
            wpool = phA.enter_context(tc.tile_pool(name="w", bufs=1))
            psA = phA.enter_context(tc.tile_pool(name="psA", bufs=4, space="PSUM"))
            psV = phA.enter_context(tc.tile_pool(name="psV", bufs=4, space="PSUM"))

            xTs = [xpool.tile([128, S], F32R, name=f"xTs{i}") for i in range(KC)]
            wq_s = [wpool.tile([128, HP], F32R, name=f"wq{i}") for i in range(KC)]
            wk_s = [wpool.tile([128, HP], F32R, name=f"wk{i}") for i in range(KC)]
            wv_s = [wpool.tile([128, HL * (HD + 1)], F32R, name=f"wv{i}") for i in range(KC)]
            # value weights first (V projection is the first consumer)
            for i in range(KC):
                eng = nc.sync if i % 2 == 0 else nc.scalar
                eng.dma_start(out=wv_s[i][:], in_=wvT.ap()[i * 128 : (i + 1) * 128, :])

            # stream x in s-stripes; project each stripe (V + QT/KT) as it lands
            for j in range(SC):
                s0 = j * SCW
                for i in range(KC):
                    eng = nc.sync if (i + j) % 2 == 0 else nc.scalar
                    eng.dma_start(
                        out=xTs[i][:, s0 : s0 + SCW],
                        in_=xT.ap()[i * 128 : (i + 1) * 128, s0 : s0 + SCW],
                    )
                if j == 0:
                    for i in range(KC):
                        eng = nc.sync if i % 2 == 0 else nc.scalar
                        eng.dma_start(out=wq_s[i][:], in_=wqT.ap()[i * 128 : (i + 1) * 128, :])
                        eng.dma_start(out=wk_s[i][:], in_=wkT.ap()[i * 128 : (i + 1) * 128, :])
                for st in range(s0 // 128, (s0 + SCW) // 128):
                    ps = psV.tile([128, HL * (HD + 1)], F32, name="ps_v", tag="ps_v")
                    for ic in range(KC):
                        nc.tensor.matmul(
                            ps[:],
                            xTs[ic][:, st * 128 : (st + 1) * 128],
                            wv_s[ic][:],
                            start=(ic == 0),
                            stop=(ic == KC - 1),
                        )
                    nc.vector.tensor_add(
                        out=Vt[st][:, :, :],
                        in0=ps[:].rearrange("p (h d) -> p h d", h=HL),
                        in1=bv_sb[:].rearrange("p (h d) -> p h d", h=HL),
                    )
                for hp in range(NPAIR):
                    for dst, wsrc, bsb in ((QT, wq_s, bq_sb), (KT, wk_s, bk_sb)):
                        ps = psA.tile([128, SCW], F32, name="ps_qk", tag="ps_qk")
                        for ic in range(KC):
                            nc.tensor.matmul(
                                ps[:],
                                wsrc[ic][:, hp * 128 : (hp + 1) * 128],
                                xTs[ic][:, s0 : s0 + SCW],
                                start=(ic == 0),
                                stop=(ic == KC - 1),
                            )
                        nc.vector.tensor_scalar_add(
                            out=dst[hp][:, s0 : s0 + SCW],
                            in0=ps[:],
                            scalar1=bsb[:, hp : hp + 1],
                        )

        # ---------------- phase B: attention ----------------
        # Heads processed in pairs: even head on PE row-strip 0-63, odd head on
        # 64-127 (concurrent via tile_position auto-derive). One ACT instruction
        # exponentiates both heads' scores. pav chunks are 1 PSUM bank wide and
        # 4-deep so the normalization chain never stalls the PE.
        QWN = 512 if S >= 512 else S   # q window per pav chunk
        with ExitStack() as phB:
            expool = phB.enter_context(tc.tile_pool(name="exps", bufs=4))
            rcppool = phB.enter_context(tc.tile_pool(name="rcp", bufs=3))
            dpool = phB.enter_context(tc.tile_pool(name="dsb", bufs=3))
            opool = phB.enter_context(tc.tile_pool(name="osb", bufs=8))
            psS = phB.enter_context(tc.tile_pool(name="psS", bufs=2, space="PSUM"))
            psAV = phB.enter_context(tc.tile_pool(name="psAV", bufs=2, space="PSUM"))
            psO = phB.enter_context(tc.tile_pool(name="psO", bufs=2, space="PSUM"))

            osb_n = QWN // 128
            osb_w = [None] * osb_n
            for qw in range(S // QWN):
                q0 = qw * QWN
                for hp in range(NPAIR):
                    pav = [
                        psAV.tile([65, QWN], F32, name=f"pav{e}", tag="pav")
                        for e in range(2)
                    ]
                    for kb in range(KB):
                        k0 = kb * 128
                        ps = psS.tile([128, 2 * QWN], F32, name="ps_s", tag="ps_s")
                        for e in range(2):
                            nc.tensor.matmul(
                                ps[:, e * QWN : (e + 1) * QWN],
                                KT[hp][e * 64 : e * 64 + 64, k0 : k0 + 128],
                                QT[hp][e * 64 : e * 64 + 64, q0 : q0 + QWN],
                                start=True,
                                stop=True,
                            )
                        es = expool.tile([128, 2 * QWN], exp_dtype, name="es", tag="es")
                        nc.scalar.activation(es[:], ps[:], AF.Exp, scale=float(HD) ** -0.5)
                        for e in range(2):
                            nc.tensor.matmul(
                                pav[e][:, :],
                                Vt[kb][:, 2 * hp + e, :],
                                es[:, e * QWN : (e + 1) * QWN],
                                start=(kb == 0),
                                stop=(kb == KB - 1),
                            )
                    # normalize both heads of this q window; stage attnU out of
                    # PSUM immediately so pav recycles fast (bufs=2)
                    for e in range(2):
                        h = 2 * hp + e
                        d_sb = dpool.tile([65, QWN], F32, name="d_sb", tag="d_sb")
                        attnU = dpool.tile([64, QWN], F32, name="attnU", tag="attnU")
                        den_rsh = dpool.tile([128, QWN // 128], F32, name="den_rsh", tag="den_rsh")
                        rcpb = rcppool.tile([64, QWN], F32, name="rcpb", tag="rcpb")
                        nc.vector.tensor_copy(d_sb[64:65, :], pav[e][64:65, :])
                        nc.vector.tensor_copy(attnU[:, :], pav[e][0:64, :])
                        nc.gpsimd.dma_start(
                            out=den_dram.ap()[h, q0 : q0 + QWN], in_=d_sb[64:65, :]
                        )
                        nc.gpsimd.dma_start(
                            out=den_rsh[:, :],
                            in_=den_dram.ap()[h, q0 : q0 + QWN].rearrange(
                                "(p i) -> p i", p=128
                            ),
                        )
                        nc.vector.reciprocal(out=den_rsh[:, :], in_=den_rsh[:, :])
                        nc.gpsimd.dma_start(
                            out=rcp_dram.ap()[h, q0 : q0 + QWN].rearrange(
                                "(p i) -> p i", p=128
                            ),
                            in_=den_rsh[:, :],
                        )
                        rcp_bc = bass.AP(
                            tensor=rcp_dram.ap().tensor,
                            offset=h * S + q0,
                            ap=[[0, 64], [1, QWN]],
                        )
                        nc.gpsimd.dma_start(out=rcpb[:, :], in_=rcp_bc)
                        if e == 0:
                            nc.vector.tensor_mul(
                                out=attnP[hp][0:64, q0 : q0 + QWN],
                                in0=attnU[:, :],
                                in1=rcpb[:, :],
                            )
                        else:
                            todd = dpool.tile([64, QWN], F32R, name="todd", tag="todd")
                            nc.vector.tensor_mul(
                                out=todd[:, :], in0=attnU[:, :], in1=rcpb[:, :]
                            )
                            nc.sync.dma_start(
                                out=attnP[hp][64:128, q0 : q0 + QWN], in_=todd[:, :]
                            )
                    # output projection contribution of this pair for the q
                    # window (emitted right after the pair's attnP is ready so
                    # it fills the next block's PE slack); pair contributions
                    # accumulate in SBUF
                    for qt in range(q0 // 128, (q0 + QWN) // 128):
                        if hp == 0:
                            osb_w[qt % osb_n] = opool.tile(
                                [128, D], F32, name="osb", tag="osb"
                            )
                        osb = osb_w[qt % osb_n]
                        for oc in range(D // 512):
                            po = psO.tile([128, 512], F32, name="po", tag="po")
                            nc.tensor.matmul(
                                po[:],
                                attnP[hp][:, qt * 128 : (qt + 1) * 128],
                                wo_s[hp][:, oc * 512 : (oc + 1) * 512],
                                start=True,
                                stop=True,
                            )
                            if hp == 0:
                                nc.vector.tensor_copy(
                                    osb[:, oc * 512 : (oc + 1) * 512], po[:]
                                )
                            else:
                                nc.vector.tensor_add(
                                    out=osb[:, oc * 512 : (oc + 1) * 512],
                                    in0=osb[:, oc * 512 : (oc + 1) * 512],
                                    in1=po[:],
                                )
                        if hp == NPAIR - 1:
                            nc.sync.dma_start(
                                out=outP.ap()[qt * 128 : (qt + 1) * 128, :], in_=osb[:]
                            )

    nc.compile()
    return nc


_NC = None


def _get_nc():
    global _NC
    if _NC is None:
        _install_neff_cache()
        _NC = build_mha_core()
    return _NC


def _wv_aug(w_v, hs):
    """[D, HL*(HD+1)]: per head its 64 value columns + a zero column (the V
    projection's bias broadcast turns it into the softmax-denominator ones)."""
    out = np.zeros((D, HL * (HD + 1)), np.float32)
    for h in range(HL):
        out[:, h * (HD + 1) : h * (HD + 1) + HD] = w_v[hs + h * HD : hs + (h + 1) * HD].T
    return out


def _bv_aug(b_v, hs):
    out = np.zeros(HL * (HD + 1), np.float32)
    for h in range(HL):
        out[h * (HD + 1) : h * (HD + 1) + HD] = b_v[hs + h * HD : hs + (h + 1) * HD]
        out[h * (HD + 1) + HD] = 1.0
    return out


def _make_in_maps(x, w_q, b_q, w_k, b_k, w_v, b_v, w_o):
    f32 = np.float32
    in_maps = []
    for core in range(N_CORES):
        b, hg = core // (N_CORES // B), core % (N_CORES // B)
        hs, he = hg * HP, (hg + 1) * HP
        in_maps.append(
            {
                "xT": np.ascontiguousarray(x[b].T, dtype=f32),
                "wqT": np.ascontiguousarray(w_q[hs:he].T, dtype=f32),
                "wkT": np.ascontiguousarray(w_k[hs:he].T, dtype=f32),
                "wvT": _wv_aug(w_v, hs),
                "woT": np.ascontiguousarray(w_o[:, hs:he].T, dtype=f32),
                "bq": np.asarray(b_q[hs:he], dtype=f32),
                "bk": np.asarray(b_k[hs:he], dtype=f32),
                "bv": _bv_aug(b_v, hs),
            }
        )
    return in_maps


def _unshard(results, b_o):
    gpb = N_CORES // B
    out = np.empty((B, S, D), np.float32)
    bo = np.asarray(b_o, dtype=np.float64)
    for b in range(B):
        acc = np.zeros((S, D), np.float64)
        for g in range(gpb):
            acc += results[b * gpb + g]["outP"]
        out[b] = (acc + bo).astype(np.float32)
    return out


def kernel(x, w_q, b_q, w_k, b_k, w_v, b_v, w_o, b_o):
    x = np.asarray(x)
    nc = _get_nc()
    in_maps = _make_in_maps(x, w_q, b_q, w_k, b_k, w_v, b_v, w_o)
    res = run_bass_kernel_spmd(nc, in_maps, core_ids=list(range(N_CORES)))
    return _unshard(res.results, b_o)


# revision 16
# speedup vs baseline: 1.1023x; 1.1023x over previous
"""TRN2 Bass kernel for nn_MultiHeadSelfAttention (B=2, S=2048, D=1024, H=16).

Sharding: 8 NeuronCores = 2 batches x 4 head-groups (4 heads each).
Each core computes its batch's Q/K/V projections for its 4 heads, the
attention, and a partial output projection; the host sums the 4 partials
per batch and adds b_o.

Per-core pipeline (all matmuls in float32r = TF32-like, full PE rate):
  - host feeds x[b].T and pre-transposed weight slices (no on-chip transposes)
  - QT/KT projections land with head-dim on partitions; V lands natural
    [s, head, 64] with a fused ones column (gives softmax denominators free)
  - scores computed transposed [keys, q] per 128-key block; exp on ScalarE
    with the 1/sqrt(64) scale fused; no max subtraction (scores bounded ~9)
  - attnU^T [65, S] accumulates over key blocks in PSUM; row 64 = denominator
  - normalize via reciprocal + DRAM-bounce partition broadcast
  - output projection accumulates all 4 heads per PSUM bank
"""

import hashlib
import os
import shutil
from contextlib import ExitStack

import numpy as np

import concourse.bass as bass
import concourse.bacc as bacc
import concourse.mybir as mybir
import concourse.tile as tile
from concourse import bass_utils, bass2jax
from concourse.bass_utils import run_bass_kernel_spmd

F32 = mybir.dt.float32
F32R = mybir.dt.float32r
AF = mybir.ActivationFunctionType

B, S, D, NH, HD = 2, 2048, 1024, 16, 64
N_CORES = 8
HL = NH // (N_CORES // B)  # 4 local heads per core
HP = HL * HD               # 256 local head dims

_NEFF_CACHE = os.path.expanduser("~/.cache/bass-neff-cache")


def _install_neff_cache():
    """Content-addressed disk cache around compile_bir_kernel (walrus is slow)."""
    if getattr(bass2jax, "_neff_cache_installed", False):
        return
    os.makedirs(_NEFF_CACHE, exist_ok=True)
    orig = bass2jax.compile_bir_kernel

    def cached(bir_json, tmpdir, neff_name="file.neff"):
        key = hashlib.sha256(bir_json).hexdigest()[:32]
        cpath = os.path.join(_NEFF_CACHE, key + ".neff")
        dst = os.path.join(tmpdir, neff_name)
        if os.path.exists(cpath):
            shutil.copy(cpath, dst)
            return dst
        out = orig(bir_json, tmpdir, neff_name=neff_name)
        try:
            shutil.copy(out, cpath)
        except OSError:
            pass
        return out

    bass2jax.compile_bir_kernel = cached
    bass2jax._neff_cache_installed = True


def build_mha_core(S=S, D=D, HL=HL, HD=HD, exp_dtype=F32R):
    """Build + compile the per-core MHA program."""
    HP = HL * HD
    KC = D // 128             # contraction chunks for projections
    KB = S // 128             # key blocks
    ST = S // 128             # s tiles for V projection / output
    QCH = min(1024, S)        # q chunk (scores psum free dim)
    NQH = S // QCH
    QW = min(512, QCH)
    NQQ = QCH // QW
    NPAIR = HP // 128
    SC = max(1, S // 512)
    SCW = min(512, S)

    nc = bacc.Bacc("TRN2", target_bir_lowering=False, debug=False)

    xT = nc.dram_tensor("xT", [D, S], F32R, kind="ExternalInput")
    wqT = nc.dram_tensor("wqT", [D, HP], F32R, kind="ExternalInput")
    wkT = nc.dram_tensor("wkT", [D, HP], F32R, kind="ExternalInput")
    wvT = nc.dram_tensor("wvT", [D, HL * (HD + 1)], F32R, kind="ExternalInput")
    woT = nc.dram_tensor("woT", [HP, D], F32R, kind="ExternalInput")
    bq = nc.dram_tensor("bq", [HP], F32, kind="ExternalInput")
    bk = nc.dram_tensor("bk", [HP], F32, kind="ExternalInput")
    bv = nc.dram_tensor("bv", [HL * (HD + 1)], F32, kind="ExternalInput")
    outP = nc.dram_tensor("outP", [S, D], F32, kind="ExternalOutput")
    den_dram = nc.dram_tensor("den_dram", [HL, S], F32, kind="Internal")
    rcp_dram = nc.dram_tensor("rcp_dram", [HL, S], F32, kind="Internal")

    with tile.TileContext(nc) as tc, ExitStack() as top:
        persist = top.enter_context(tc.tile_pool(name="persist", bufs=1))

        QT = [persist.tile([128, S], F32R, name=f"QT{i}") for i in range(NPAIR)]
        KT = [persist.tile([128, S], F32R, name=f"KT{i}") for i in range(NPAIR)]
        Vt = [persist.tile([128, HL, HD + 1], F32R, name=f"V{i}") for i in range(KB)]
        attnP = [persist.tile([128, S], F32R, name=f"attnP{i}") for i in range(NPAIR)]
        wo_s = [persist.tile([128, D], F32R, name=f"wo{i}") for i in range(NPAIR)]
        bq_sb = persist.tile([128, NPAIR], F32, name="bq_sb")
        bk_sb = persist.tile([128, NPAIR], F32, name="bk_sb")
        bv_sb = persist.tile([128, HL * (HD + 1)], F32, name="bv_sb")

        nc.gpsimd.dma_start(out=bq_sb[:, :], in_=bq.ap().rearrange("(hp p) -> p hp", p=128))
        nc.gpsimd.dma_start(out=bk_sb[:, :], in_=bk.ap().rearrange("(hp p) -> p hp", p=128))
        bv_bc = bass.AP(tensor=bv.ap().tensor, offset=0, ap=[[0, 128], [1, HL * (HD + 1)]])
        nc.gpsimd.dma_start(out=bv_sb[:, :], in_=bv_bc)
        for i in range(NPAIR):
            nc.gpsimd.dma_start(out=wo_s[i][:], in_=woT.ap()[i * 128 : (i + 1) * 128, :])

        # ---------------- phase A: projections ----------------
        # Loads are emitted ic-interleaved and split across both HWDGE engines
        # (SP + ACT) so projection matmuls start as soon as chunk 0 lands.
        # V is projected first (attention's kb loop needs every V tile), then
        # the Q/K pair-0 tiles so the first attention block starts early.
        with ExitStack() as phA:
            xpool = phA.enter_context(tc.tile_pool(name="xTp", bufs=1))
            wpool = phA.enter_context(tc.tile_pool(name="w", bufs=1))
            psA = phA.enter_context(tc.tile_pool(name="psA", bufs=4, space="PSUM"))
            psV = phA.enter_context(tc.tile_pool(name="psV", bufs=4, space="PSUM"))

            xTs = [xpool.tile([128, S], F32R, name=f"xTs{i}") for i in range(KC)]
            wq_s = [wpool.tile([128, HP], F32R, name=f"wq{i}") for i in range(KC)]
            wk_s = [wpool.tile([128, HP], F32R, name=f"wk{i}") for i in range(KC)]
            wv_s = [wpool.tile([128, HL * (HD + 1)], F32R, name=f"wv{i}") for i in range(KC)]
            # value weights first (V projection is the first consumer)
            for i in range(KC):
                eng = nc.sync if i % 2 == 0 else nc.scalar
                eng.dma_start(out=wv_s[i][:], in_=wvT.ap()[i * 128 : (i + 1) * 128, :])

            # stream x in s-stripes; project each stripe (V + QT/KT) as it lands
            for j in range(SC):
                s0 = j * SCW
                for i in range(KC):
                    eng = nc.sync if (i + j) % 2 == 0 else nc.scalar
                    eng.dma_start(
                        out=xTs[i][:, s0 : s0 + SCW],
                        in_=xT.ap()[i * 128 : (i + 1) * 128, s0 : s0 + SCW],
                    )
                if j == 0:
                    for i in range(KC):
                        eng = nc.sync if i % 2 == 0 else nc.scalar
                        eng.dma_start(out=wq_s[i][:], in_=wqT.ap()[i * 128 : (i + 1) * 128, :])
                        eng.dma_start(out=wk_s[i][:], in_=wkT.ap()[i * 128 : (i + 1) * 128, :])
                for st in range(s0 // 128, (s0 + SCW) // 128):
                    ps = psV.tile([128, HL * (HD + 1)], F32, name="ps_v", tag="ps_v")
                    for ic in range(KC):
                        nc.tensor.matmul(
                            ps[:],
                            xTs[ic][:, st * 128 : (st + 1) * 128],
                            wv_s[ic][:],
                            start=(ic == 0),
                            stop=(ic == KC - 1),
                        )
                    nc.vector.tensor_add(
                        out=Vt[st][:, :, :],
                        in0=ps[:].rearrange("p (h d) -> p h d", h=HL),
                        in1=bv_sb[:].rearrange("p (h d) -> p h d", h=HL),
                    )
                for hp in range(NPAIR):
                    for dst, wsrc, bsb in ((QT, wq_s, bq_sb), (KT, wk_s, bk_sb)):
                        ps = psA.tile([128, SCW], F32, name="ps_qk", tag="ps_qk")
                        for ic in range(KC):
                            nc.tensor.matmul(
                                ps[:],
                                wsrc[ic][:, hp * 128 : (hp + 1) * 128],
                                xTs[ic][:, s0 : s0 + SCW],
                                start=(ic == 0),
                                stop=(ic == KC - 1),
                            )
                        nc.vector.tensor_scalar_add(
                            out=dst[hp][:, s0 : s0 + SCW],
                            in0=ps[:],
                            scalar1=bsb[:, hp : hp + 1],
                        )

            warm_sb = wpool.tile([128, 16], F32, name="warm_sb")
            ps_w = psA.tile([128, 512], F32, name="ps_warm", tag="ps_qk")
            for i in range(18):
                nc.tensor.matmul(
                    ps_w[:],
                    wq_s[i % KC][:, 0:128],
                    xTs[i % KC][:, 0:512],
                    start=(i == 0),
                    stop=(i == 17),
                )
            nc.vector.tensor_copy(warm_sb[:, :], ps_w[:, 0:16])

        # ---------------- phase B: attention ----------------
        # Heads processed in pairs: even head on PE row-strip 0-63, odd head on
        # 64-127 (concurrent via tile_position auto-derive). One ACT instruction
        # exponentiates both heads' scores. pav chunks are 1 PSUM bank wide and
        # 4-deep so the normalization chain never stalls the PE.
        QWN = 512 if S >= 512 else S   # q window per pav chunk
        with ExitStack() as phB:
            expool = phB.enter_context(tc.tile_pool(name="exps", bufs=4))
            rcppool = phB.enter_context(tc.tile_pool(name="rcp", bufs=3))
            dpool = phB.enter_context(tc.tile_pool(name="dsb", bufs=3))
            opool = phB.enter_context(tc.tile_pool(name="osb", bufs=8))
            psS = phB.enter_context(tc.tile_pool(name="psS", bufs=2, space="PSUM"))
            psAV = phB.enter_context(tc.tile_pool(name="psAV", bufs=2, space="PSUM"))
            psO = phB.enter_context(tc.tile_pool(name="psO", bufs=2, space="PSUM"))

            osb_n = QWN // 128
            osb_w = [None] * osb_n
            for qw in range(S // QWN):
                q0 = qw * QWN
                for hp in range(NPAIR):
                    pav = [
                        psAV.tile([65, QWN], F32, name=f"pav{e}", tag="pav")
                        for e in range(2)
                    ]
                    for kb in range(KB):
                        k0 = kb * 128
                        ps = psS.tile([128, 2 * QWN], F32, name="ps_s", tag="ps_s")
                        for e in range(2):
                            nc.tensor.matmul(
                                ps[:, e * QWN : (e + 1) * QWN],
                                KT[hp][e * 64 : e * 64 + 64, k0 : k0 + 128],
                                QT[hp][e * 64 : e * 64 + 64, q0 : q0 + QWN],
                                start=True,
                                stop=True,
                            )
                        es = expool.tile([128, 2 * QWN], exp_dtype, name="es", tag="es")
                        nc.scalar.activation(es[:], ps[:], AF.Exp, scale=float(HD) ** -0.5)
                        for e in range(2):
                            nc.tensor.matmul(
                                pav[e][:, :],
                                Vt[kb][:, 2 * hp + e, :],
                                es[:, e * QWN : (e + 1) * QWN],
                                start=(kb == 0),
                                stop=(kb == KB - 1),
                            )
                    # normalize both heads of this q window; stage attnU out of
                    # PSUM immediately so pav recycles fast (bufs=2)
                    for e in range(2):
                        h = 2 * hp + e
                        d_sb = dpool.tile([65, QWN], F32, name="d_sb", tag="d_sb")
                        attnU = dpool.tile([64, QWN], F32, name="attnU", tag="attnU")
                        den_rsh = dpool.tile([128, QWN // 128], F32, name="den_rsh", tag="den_rsh")
                        rcpb = rcppool.tile([64, QWN], F32, name="rcpb", tag="rcpb")
                        nc.vector.tensor_copy(d_sb[64:65, :], pav[e][64:65, :])
                        nc.vector.tensor_copy(attnU[:, :], pav[e][0:64, :])
                        nc.sync.dma_start(
                            out=den_dram.ap()[h, q0 : q0 + QWN], in_=d_sb[64:65, :]
                        )
                        nc.sync.dma_start(
                            out=den_rsh[:, :],
                            in_=den_dram.ap()[h, q0 : q0 + QWN].rearrange(
                                "(p i) -> p i", p=128
                            ),
                        )
                        nc.vector.reciprocal(out=den_rsh[:, :], in_=den_rsh[:, :])
                        nc.sync.dma_start(
                            out=rcp_dram.ap()[h, q0 : q0 + QWN].rearrange(
                                "(p i) -> p i", p=128
                            ),
                            in_=den_rsh[:, :],
                        )
                        rcp_bc = bass.AP(
                            tensor=rcp_dram.ap().tensor,
                            offset=h * S + q0,
                            ap=[[0, 64], [1, QWN]],
                        )
                        nc.sync.dma_start(out=rcpb[:, :], in_=rcp_bc)
                        if e == 0:
                            nc.vector.tensor_mul(
                                out=attnP[hp][0:64, q0 : q0 + QWN],
                                in0=attnU[:, :],
                                in1=rcpb[:, :],
                            )
                        else:
                            todd = dpool.tile([64, QWN], F32R, name="todd", tag="todd")
                            nc.vector.tensor_mul(
                                out=todd[:, :], in0=attnU[:, :], in1=rcpb[:, :]
                            )
                            nc.sync.dma_start(
                                out=attnP[hp][64:128, q0 : q0 + QWN], in_=todd[:, :]
                            )
                    # output projection contribution of this pair for the q
                    # window (emitted right after the pair's attnP is ready so
                    # it fills the next block's PE slack); pair contributions
                    # accumulate in SBUF
                    for qt in range(q0 // 128, (q0 + QWN) // 128):
                        if hp == 0:
                            osb_w[qt % osb_n] = opool.tile(
                                [128, D], F32, name="osb", tag="osb"
                            )
                        osb = osb_w[qt % osb_n]
                        for oc in range(D // 512):
                            po = psO.tile([128, 512], F32, name="po", tag="po")
                            nc.tensor.matmul(
                                po[:],
                                attnP[hp][:, qt * 128 : (qt + 1) * 128],
                                wo_s[hp][:, oc * 512 : (oc + 1) * 512],
                                start=True,
                                stop=True,
                            )
                            if hp == 0:
                                nc.vector.tensor_copy(
                                    osb[:, oc * 512 : (oc + 1) * 512], po[:]
                                )
                            else:
                                nc.vector.tensor_add(
                                    out=osb[:, oc * 512 : (oc + 1) * 512],
                                    in0=osb[:, oc * 512 : (oc + 1) * 512],
                                    in1=po[:],
                                )
                        if hp == NPAIR - 1:
                            nc.sync.dma_start(
                                out=outP.ap()[qt * 128 : (qt + 1) * 128, :], in_=osb[:]
                            )

    nc.compile()
    return nc


_NC = None


def _get_nc():
    global _NC
    if _NC is None:
        _install_neff_cache()
        _NC = build_mha_core()
    return _NC


def _wv_aug(w_v, hs):
    """[D, HL*(HD+1)]: per head its 64 value columns + a zero column (the V
    projection's bias broadcast turns it into the softmax-denominator ones)."""
    out = np.zeros((D, HL * (HD + 1)), np.float32)
    for h in range(HL):
        out[:, h * (HD + 1) : h * (HD + 1) + HD] = w_v[hs + h * HD : hs + (h + 1) * HD].T
    return out


def _bv_aug(b_v, hs):
    out = np.zeros(HL * (HD + 1), np.float32)
    for h in range(HL):
        out[h * (HD + 1) : h * (HD + 1) + HD] = b_v[hs + h * HD : hs + (h + 1) * HD]
        out[h * (HD + 1) + HD] = 1.0
    return out


def _make_in_maps(x, w_q, b_q, w_k, b_k, w_v, b_v, w_o):
    f32 = np.float32
    in_maps = []
    for core in range(N_CORES):
        b, hg = core // (N_CORES // B), core % (N_CORES // B)
        hs, he = hg * HP, (hg + 1) * HP
        in_maps.append(
            {
                "xT": np.ascontiguousarray(x[b].T, dtype=f32),
                "wqT": np.ascontiguousarray(w_q[hs:he].T, dtype=f32),
                "wkT": np.ascontiguousarray(w_k[hs:he].T, dtype=f32),
                "wvT": _wv_aug(w_v, hs),
                "woT": np.ascontiguousarray(w_o[:, hs:he].T, dtype=f32),
                "bq": np.asarray(b_q[hs:he], dtype=f32),
                "bk": np.asarray(b_k[hs:he], dtype=f32),
                "bv": _bv_aug(b_v, hs),
            }
        )
    return in_maps


def _unshard(results, b_o):
    gpb = N_CORES // B
    out = np.empty((B, S, D), np.float32)
    bo = np.asarray(b_o, dtype=np.float64)
    for b in range(B):
        acc = np.zeros((S, D), np.float64)
        for g in range(gpb):
            acc += results[b * gpb + g]["outP"]
        out[b] = (acc + bo).astype(np.float32)
    return out


def kernel(x, w_q, b_q, w_k, b_k, w_v, b_v, w_o, b_o):
    x = np.asarray(x)
    nc = _get_nc()
    in_maps = _make_in_maps(x, w_q, b_q, w_k, b_k, w_v, b_v, w_o)
    res = run_bass_kernel_spmd(nc, in_maps, core_ids=list(range(N_CORES)))
    return _unshard(res.results, b_o)
